# revision 1
# baseline (speedup 1.0000x reference)
"""M2MRF module as a two-GEMM chained Bass kernel on 8 TRN2 NeuronCores.

Math (per batch b of 4):
    cols = unfold(x[b], k=4, s=4)            # [1024, 16384]
    y1   = W1 @ cols + b1                    # [1024, 16384]
    y2   = W2 @ y1 + b2                      # [256, 16384]
    out[b] = fold(y2, k=2, s=2)              # [64, 256, 256]

Sharding: 8 cores = 4 batches x 2 L-halves (L = 16384 patch positions).
Each core runs GEMM1 (1024x1024x8192) + GEMM2 (256x1024x8192) in bf16
with fp32 PSUM accumulation. Unfold/fold are pure data-movement and run
on the host; the device sees contiguous [K, L] operands resident in SBUF.
"""
import sys

sys.path.insert(0, "/opt/trn_rl_repo")

import numpy as np
import ml_dtypes

import concourse.bass as bass
import concourse.bacc as bacc
import concourse.mybir as mybir
import concourse.tile as tile
from concourse.bass_utils import run_bass_kernel_spmd

P = 128
NT = 512            # free-dim tile (one PSUM bank of fp32)
LSH = 8192          # L per core
NTILES = LSH // NT  # 16
KC = 8              # 1024 / 128 contraction chunks
FC = 1024
COUT = 256

_BF16 = ml_dtypes.bfloat16


def _build_nc(ntiles=NTILES):
    nc = bacc.Bacc("TRN2", target_bir_lowering=False)
    xc_dram = [
        nc.dram_tensor(f"xc{k}", [P, LSH], mybir.dt.bfloat16, kind="ExternalInput")
        for k in range(KC)
    ]
    w1_dram = nc.dram_tensor("w1t", [KC, P, FC], mybir.dt.bfloat16, kind="ExternalInput")  # [m, p, k*128+j]
    w2_dram = nc.dram_tensor("w2t", [KC, P, COUT], mybir.dt.bfloat16, kind="ExternalInput")
    y2_dram = nc.dram_tensor("y2", [2, P, LSH], mybir.dt.float32, kind="ExternalOutput")

    with tile.TileContext(nc) as tc:
        with (
            tc.tile_pool(name="resident", bufs=1) as res,
            tc.tile_pool(name="work", bufs=2) as work,
            tc.tile_pool(name="outp", bufs=3) as outp,
            tc.tile_pool(name="ps1", bufs=4, space="PSUM") as ps1,
            tc.tile_pool(name="ps2", bufs=2, space="PSUM") as ps2,
        ):
            w1_sb = res.tile([P, KC, FC], mybir.dt.bfloat16, tag="w1")
            w2_sb = res.tile([P, KC, COUT], mybir.dt.bfloat16, tag="w2")
            xc_sb = [
                res.tile([P, LSH], mybir.dt.bfloat16, tag=f"xc{k}", name=f"xc{k}")
                for k in range(KC)
            ]
            # Issue order tracks first use: the opening m-group of tile 0 needs
            # only W1's m=0 slice plus the head slice of every x chunk.
            nc.sync.dma_start(w1_sb[:, 0, :], w1_dram.ap()[0])
            hsl = slice(0, LSH // 8)
            for k in range(KC):
                nc.sync.dma_start(xc_sb[k][:, hsl], xc_dram[k].ap()[:, hsl])
            for m in range(1, KC):
                nc.sync.dma_start(w1_sb[:, m, :], w1_dram.ap()[m])
            nc.sync.dma_start(w2_sb[:], w2_dram.ap().rearrange("k p m -> p k m"))
            for h in range(1, 8):
                sl = slice(h * (LSH // 8), (h + 1) * (LSH // 8))
                for k in range(KC):
                    nc.sync.dma_start(xc_sb[k][:, sl], xc_dram[k].ap()[:, sl])

            for nt in range(ntiles):
                nsl = slice(nt * NT, (nt + 1) * NT)
                y1_sb = work.tile([P, KC, NT], mybir.dt.bfloat16, tag="y1")
                # GEMM1: y1[m,:] = sum_k W1T[k,:,m]^T @ xc[k][:, nsl]
                for m in range(KC):
                    pt = ps1.tile([P, NT], mybir.dt.float32, tag="ps1")
                    for k in range(KC):
                        nc.tensor.matmul(
                            pt[:],
                            w1_sb[:, m, k * P:(k + 1) * P],
                            xc_sb[k][:, nsl],
                            start=(k == 0),
                            stop=(k == KC - 1),
                        )
                    nc.vector.tensor_copy(y1_sb[:, m, :], pt[:])
                # GEMM2: y2[m2,:] = sum_k W2T[k,:,m2]^T @ y1[k,:]
                o_sb = outp.tile([P, 2, NT], mybir.dt.float32, tag="o")
                for m2 in range(2):
                    pt2 = ps2.tile([P, NT], mybir.dt.float32, tag="ps2")
                    for k in range(KC):
                        nc.tensor.matmul(
                            pt2[:],
                            w2_sb[:, k, m2 * P:(m2 + 1) * P],
                            y1_sb[:, k, :],
                            start=(k == 0),
                            stop=(k == KC - 1),
                        )
                    nc.any.tensor_copy(out=o_sb[:, m2, :], in_=pt2[:])
                    nc.sync.dma_start(y2_dram.ap()[m2, :, nsl], o_sb[:, m2, :])

    nc.finalize()
    return nc


_NC_CACHE = None


def kernel(x, W1, b1, W2, b2):
    global _NC_CACHE
    x = np.asarray(x)
    W1, b1 = np.asarray(W1), np.asarray(b1)
    W2, b2 = np.asarray(W2), np.asarray(b2)
    n, c, h, w = x.shape  # 4, 64, 512, 512

    # ---- host unfold: cols[b, c*16+kh*4+kw, ph*128+pw] = x[b,c,ph*4+kh,pw*4+kw]
    xb = x.astype(_BF16)
    cols = xb.reshape(n, c, 128, 4, 128, 4).transpose(0, 1, 3, 5, 2, 4)
    cols = np.ascontiguousarray(cols).reshape(n, 1024, 16384)

    w1t = np.ascontiguousarray(
        W1.astype(_BF16).reshape(KC, P, KC, P).transpose(0, 3, 2, 1)
    ).reshape(KC, P, FC)
    w2t = np.ascontiguousarray(W2.T.astype(_BF16)).reshape(KC, P, COUT)

    if _NC_CACHE is None:
        _NC_CACHE = _build_nc()
    nc = _NC_CACHE

    in_maps = []
    for core in range(8):
        b, half = core // 2, core % 2
        xc = np.ascontiguousarray(
            cols[b, :, half * LSH:(half + 1) * LSH]
        ).reshape(KC, P, LSH)
        m = {f"xc{k}": xc[k] for k in range(KC)}
        m["w1t"] = w1t
        m["w2t"] = w2t
        in_maps.append(m)

    res = run_bass_kernel_spmd(nc, in_maps, core_ids=list(range(8)))

    # ---- gather + fold on host
    y2 = np.empty((n, COUT, 16384), dtype=np.float32)
    for core in range(8):
        b, half = core // 2, core % 2
        y2[b, :, half * LSH:(half + 1) * LSH] = (
            res.results[core]["y2"].reshape(COUT, LSH)
        )

    # bias epilogue (b1/b2 are zeros in this problem; exact otherwise)
    v = W2.astype(np.float64) @ b1.astype(np.float64) + b2.astype(np.float64)
    if np.any(v):
        y2 += v.astype(np.float32)[None, :, None]

    out = y2.reshape(n, c, 2, 2, 128, 128).transpose(0, 1, 4, 2, 5, 3)
    return np.ascontiguousarray(out).reshape(n, c, 256, 256)



# revision 8
# speedup vs baseline: 4.1053x; 4.1053x over previous
"""M2MRF module as a single fused-GEMM Bass kernel on 8 TRN2 NeuronCores.

The reference is two chained 1x1 convs with NO nonlinearity between them:
    y2 = W2 @ (W1 @ cols + b1) + b2 = (W2@W1) @ cols + (W2@b1 + b2)
so the device work collapses to one GEMM with the fused matrix
M = W2@W1 [256, 1024] (5.2x fewer FLOPs than the two-GEMM chain).

Math (per batch b of 4):
    cols = unfold(x[b], k=4, s=4)            # [1024, 16384]
    y2   = M @ cols                          # [256, 16384]
    out[b] = fold(y2 + v, k=2, s=2)          # [64, 256, 256], v = W2@b1+b2

Sharding: 8 cores = 4 batches x 2 L-halves (L = 16384 patch positions).
Each core runs one GEMM (256x1024x8192) in bf16 with fp32 PSUM
accumulation, streaming L-tiles of 512 through SBUF. Unfold/fold are
pure data movement and run on the host.
"""
import sys

sys.path.insert(0, "/opt/trn_rl_repo")

import numpy as np
import ml_dtypes

import concourse.bass as bass
import concourse.bacc as bacc
import concourse.mybir as mybir
import concourse.tile as tile
from concourse.bass_utils import run_bass_kernel_spmd

P = 128
NT = 512            # free-dim tile (one PSUM bank of fp32)
LSH = 8192          # L per core
NTILES = LSH // NT  # 16
KC = 8              # 1024 / 128 contraction chunks
COUT = 256
OGRP = 2            # L-tiles per output DMA

_BF16 = ml_dtypes.bfloat16


# Warmup row counts: climb the PE p-state ramp (0.65/1.2 GHz until 3us of
# continuous busy) on junk data, sized to end right as the first real
# operands land (~5.2us), with a short-matmul cushion against a gap reset.
WARM_PLAN = [512] * 8 + [128] * 3


def _build_nc(ntiles=NTILES):
    nc = bacc.Bacc("TRN2", target_bir_lowering=False)
    xc_dram = nc.dram_tensor("xc", [P, KC, LSH], mybir.dt.bfloat16, kind="ExternalInput")
    wt_dram = nc.dram_tensor("wt", [P, KC, COUT], mybir.dt.bfloat16, kind="ExternalInput")
    y2_dram = nc.dram_tensor("y2", [P, 2, LSH], mybir.dt.bfloat16, kind="ExternalOutput")

    with tile.TileContext(nc) as tc:
        with (
            tc.tile_pool(name="resident", bufs=1) as res,
            tc.tile_pool(name="xin", bufs=6) as xin,
            tc.tile_pool(name="outp", bufs=3) as outp,
            tc.tile_pool(name="ps", bufs=4, space="PSUM") as ps,
            tc.tile_pool(name="wps", bufs=1, space="PSUM") as wps,
        ):
            wt_sb = res.tile([P, KC, COUT], mybir.dt.bfloat16, tag="wt")

            # Dep-free warmup matmuls on a zeroed tile: climb the PE p-state
            # ramp before real data lands so real matmuls price at full clock.
            warm = res.tile([P, NT], mybir.dt.bfloat16, tag="warm")
            nc.vector.memset(warm[:], 0)
            wpt = wps.tile([P, NT], mybir.dt.float32, tag="wps")
            for rows in WARM_PLAN:
                nc.tensor.matmul(wpt[:, 0:rows], warm[:, 0:P], warm[:, 0:rows], start=True, stop=True)

            # DMA issue order tracks first use: w0, xt0, then the w tail.
            nc.sync.dma_start(wt_sb[:, 0, :], wt_dram.ap()[:, 0, :])

            o_sb = None
            for nt in range(ntiles):
                nsl = slice(nt * NT, (nt + 1) * NT)
                xt = xin.tile([P, KC, NT], mybir.dt.bfloat16, tag="xt")
                nc.sync.dma_start(xt[:], xc_dram.ap()[:, :, nsl])
                if nt == 0:
                    for k in range(1, KC):
                        nc.sync.dma_start(wt_sb[:, k, :], wt_dram.ap()[:, k, :])
                g = nt % OGRP
                if g == 0:
                    o_sb = outp.tile([P, 2, OGRP * NT], mybir.dt.bfloat16, tag="o")
                last = nt == ntiles - 1
                for m2 in range(2):
                    pt = ps.tile([P, NT], mybir.dt.float32, tag="ps")
                    for k in range(KC):
                        nc.tensor.matmul(
                            pt[:],
                            wt_sb[:, k, m2 * P:(m2 + 1) * P],
                            xt[:, k, :],
                            start=(k == 0),
                            stop=(k == KC - 1),
                        )
                    dst = o_sb[:, m2, g * NT:(g + 1) * NT]
                    if m2 == 0:
                        nc.vector.tensor_copy(out=dst, in_=pt[:])
                    else:
                        nc.scalar.copy(out=dst, in_=pt[:])
                    if last:
                        # Tail trim: flush each m-half as soon as its copy
                        # lands, on the lighter HWDGE path (SP is idle here).
                        osl = slice((nt - g) * NT, (nt + 1) * NT)
                        nc.sync.dma_start(
                            y2_dram.ap()[:, m2, osl], o_sb[:, m2, :]
                        )
                if not last and g == OGRP - 1:
                    osl = slice((nt - g) * NT, (nt + 1) * NT)
                    nc.gpsimd.dma_start(y2_dram.ap()[:, :, osl], o_sb[:])

    nc.finalize()
    return nc


_NC_CACHE = None


def kernel(x, W1, b1, W2, b2):
    global _NC_CACHE
    x = np.asarray(x)
    W1, b1 = np.asarray(W1), np.asarray(b1)
    W2, b2 = np.asarray(W2), np.asarray(b2)
    n, c, h, w = x.shape  # 4, 64, 512, 512

    # ---- host: fuse the two pointwise convs into one matrix
    M = (W2.astype(np.float64) @ W1.astype(np.float64)).astype(np.float32)
    # wt[p, k, m] = M[m, k*128+p]
    wt = np.ascontiguousarray(
        M.T.astype(_BF16).reshape(KC, P, COUT).transpose(1, 0, 2)
    )

    # ---- host unfold: cols[b, c*16+kh*4+kw, ph*128+pw] = x[b,c,ph*4+kh,pw*4+kw]
    xb = x.astype(_BF16)
    cols = xb.reshape(n, c, 128, 4, 128, 4).transpose(0, 1, 3, 5, 2, 4)
    cols = np.ascontiguousarray(cols).reshape(n, 1024, 16384)

    if _NC_CACHE is None:
        _NC_CACHE = _build_nc()
    nc = _NC_CACHE

    in_maps = []
    for core in range(8):
        b, half = core // 2, core % 2
        xc = np.ascontiguousarray(
            cols[b].reshape(KC, P, 2 * LSH)[:, :, half * LSH:(half + 1) * LSH]
            .transpose(1, 0, 2)
        )
        in_maps.append({"xc": xc, "wt": wt})

    res = run_bass_kernel_spmd(nc, in_maps, core_ids=list(range(8)))

    # ---- gather + fold on host
    y2 = np.empty((n, COUT, 16384), dtype=np.float32)
    for core in range(8):
        b, half = core // 2, core % 2
        r = res.results[core]["y2"]  # [P, 2, LSH] bf16
        y2[b, :, half * LSH:(half + 1) * LSH] = (
            r.transpose(1, 0, 2).reshape(COUT, LSH).astype(np.float32)
        )

    # bias epilogue (b1/b2 are zeros in this problem; exact otherwise)
    v = W2.astype(np.float64) @ b1.astype(np.float64) + b2.astype(np.float64)
    if np.any(v):
        y2 += v.astype(np.float32)[None, :, None]

    out = y2.reshape(n, c, 2, 2, 128, 128).transpose(0, 1, 4, 2, 5, 3)
    return np.ascontiguousarray(out).reshape(n, c, 256, 256)


# revision 25
# speedup vs baseline: 4.4243x; 1.0777x over previous
"""M2MRF module as a single fused-GEMM Bass kernel on 8 TRN2 NeuronCores.

The reference is two chained 1x1 convs with NO nonlinearity between them:
    y2 = W2 @ (W1 @ cols + b1) + b2 = (W2@W1) @ cols + (W2@b1 + b2)
so the device work collapses to one GEMM with the fused matrix
M = W2@W1 [256, 1024] (5.2x fewer FLOPs than the two-GEMM chain).

Math (per batch b of 4):
    cols = unfold(x[b], k=4, s=4)            # [1024, 16384]
    y2   = M @ cols                          # [256, 16384]
    out[b] = fold(y2 + v, k=2, s=2)          # [64, 256, 256], v = W2@b1+b2

Sharding: 8 cores = 4 batches x 2 L-halves (L = 16384 patch positions).
Each core runs one GEMM (256x1024x8192) in bf16 with fp32 PSUM
accumulation, streaming L-tiles of 512 through SBUF. Unfold/fold are
pure data movement and run on the host.
"""
import sys

sys.path.insert(0, "/opt/trn_rl_repo")

import numpy as np
import ml_dtypes

import concourse.bass as bass
import concourse.bacc as bacc
import concourse.mybir as mybir
import concourse.tile as tile
from concourse.bass_utils import run_bass_kernel_spmd

P = 128
NT = 512            # free-dim tile (one PSUM bank of fp32)
LSH = 8192          # L per core
NTILES = LSH // NT  # 16
KC = 8              # 1024 / 128 contraction chunks
COUT = 256
OGRP = 2            # L-tiles per output DMA

_BF16 = ml_dtypes.bfloat16


# Warmup row counts: climb the PE p-state ramp (0.65/1.2 GHz until 3us of
# continuous busy) on junk data, sized to end right as the first real
# operands land (~5.8us), with short-matmul cushion against a gap reset.
WARM_PLAN = [512] * 9 + [128] * 4

# Column widths per compute tile: narrow early tiles move the first matmul
# earlier (supply = 5.69 ns/col vs demand = 6.67 ns/col lets later tiles
# grow without starving the PE).
TILE_PLAN = [256, 256, 320, 384, 448, 384] + [512] * 10 + [448, 320, 256]
# Output flush groups (tile indices): ~1024-col groups, short final group so
# the drain after the last matmul is minimal.
OUT_GROUPS = [[0, 1, 2, 3], [4, 5], [6, 7], [8, 9], [10, 11], [12, 13], [14, 15], [16], [17], [18]]


def _build_nc():
    nc = bacc.Bacc("TRN2", target_bir_lowering=False)
    xc_dram = nc.dram_tensor("xc", [P, KC, LSH], mybir.dt.bfloat16, kind="ExternalInput")
    wt_dram = nc.dram_tensor("wt", [P, KC, COUT], mybir.dt.bfloat16, kind="ExternalInput")
    y2_dram = nc.dram_tensor("y2", [P, 2, LSH], mybir.dt.bfloat16, kind="ExternalOutput")

    starts = [sum(TILE_PLAN[:i]) for i in range(len(TILE_PLAN))]
    grp_of = {}
    for gi, g in enumerate(OUT_GROUPS):
        for t in g:
            grp_of[t] = gi

    with tile.TileContext(nc) as tc:
        with (
            tc.tile_pool(name="resident", bufs=1) as res,
            tc.tile_pool(name="xin", bufs=14) as xin,
            tc.tile_pool(name="outp", bufs=10) as outp,
            tc.tile_pool(name="ps", bufs=4, space="PSUM") as ps,
            tc.tile_pool(name="wps", bufs=1, space="PSUM") as wps,
        ):
            wt_sb = res.tile([P, KC, COUT], mybir.dt.bfloat16, tag="wt")

            # Dep-free warmup matmuls on a zeroed tile: climb the PE p-state
            # ramp before real data lands so real matmuls price at full clock.
            warm = res.tile([P, NT], mybir.dt.bfloat16, tag="warm")
            nc.vector.memset(warm[:], 0)
            wpt = wps.tile([P, NT], mybir.dt.float32, tag="wps")
            for rows in WARM_PLAN:
                nc.tensor.matmul(wpt[:, 0:rows], warm[:, 0:P], warm[:, 0:rows], start=True, stop=True)

            o_sb = None
            o_base = 0
            for ti, width in enumerate(TILE_PLAN):
                nsl = slice(starts[ti], starts[ti] + width)
                # DMA issue order tracks first use: w, xt0, xt1, ...
                if ti == 0:
                    nc.sync.dma_start(wt_sb[:], wt_dram.ap())
                xt = xin.tile([P, KC, NT], mybir.dt.bfloat16, tag="xt")
                nc.sync.dma_start(xt[:, :, 0:width], xc_dram.ap()[:, :, nsl])

                gi = grp_of[ti]
                g_tiles = OUT_GROUPS[gi]
                if ti == g_tiles[0]:
                    o_base = starts[ti]
                    g_width = sum(TILE_PLAN[t] for t in g_tiles)
                    o_sb = outp.tile([P, 2, 1536], mybir.dt.bfloat16, tag="o")
                last_grp = gi == len(OUT_GROUPS) - 1
                off = starts[ti] - o_base
                for m2 in range(2):
                    pt = ps.tile([P, NT], mybir.dt.float32, tag="ps")
                    for k in range(KC):
                        nc.tensor.matmul(
                            pt[:, 0:width],
                            wt_sb[:, k, m2 * P:(m2 + 1) * P],
                            xt[:, k, 0:width],
                            start=(k == 0),
                            stop=(k == KC - 1),
                        )
                    dst = o_sb[:, m2, off:off + width]
                    if m2 == 0:
                        nc.vector.tensor_copy(out=dst, in_=pt[:, 0:width])
                    else:
                        nc.scalar.copy(out=dst, in_=pt[:, 0:width])
                    if last_grp and ti == g_tiles[-1]:
                        # Tail trim: flush each m-half as soon as its copy
                        # lands, on the lighter HWDGE path (SP is idle here).
                        osl = slice(o_base, starts[ti] + width)
                        gw = starts[ti] + width - o_base
                        nc.sync.dma_start(
                            y2_dram.ap()[:, m2, osl], o_sb[:, m2, 0:gw]
                        )
                if not last_grp and ti == g_tiles[-1]:
                    osl = slice(o_base, starts[ti] + width)
                    gw = starts[ti] + width - o_base
                    nc.gpsimd.dma_start(y2_dram.ap()[:, :, osl], o_sb[:, :, 0:gw])

    nc.finalize()
    return nc


_NC_CACHE = None


def kernel(x, W1, b1, W2, b2):
    global _NC_CACHE
    x = np.asarray(x)
    W1, b1 = np.asarray(W1), np.asarray(b1)
    W2, b2 = np.asarray(W2), np.asarray(b2)
    n, c, h, w = x.shape  # 4, 64, 512, 512

    # ---- host: fuse the two pointwise convs into one matrix
    M = (W2.astype(np.float64) @ W1.astype(np.float64)).astype(np.float32)
    # wt[p, k, m] = M[m, k*128+p]
    wt = np.ascontiguousarray(
        M.T.astype(_BF16).reshape(KC, P, COUT).transpose(1, 0, 2)
    )

    # ---- host unfold: cols[b, c*16+kh*4+kw, ph*128+pw] = x[b,c,ph*4+kh,pw*4+kw]
    xb = x.astype(_BF16)
    cols = xb.reshape(n, c, 128, 4, 128, 4).transpose(0, 1, 3, 5, 2, 4)
    cols = np.ascontiguousarray(cols).reshape(n, 1024, 16384)

    if _NC_CACHE is None:
        _NC_CACHE = _build_nc()
    nc = _NC_CACHE

    in_maps = []
    for core in range(8):
        b, half = core // 2, core % 2
        xc = np.ascontiguousarray(
            cols[b].reshape(KC, P, 2 * LSH)[:, :, half * LSH:(half + 1) * LSH]
            .transpose(1, 0, 2)
        )
        in_maps.append({"xc": xc, "wt": wt})

    res = run_bass_kernel_spmd(nc, in_maps, core_ids=list(range(8)))

    # ---- gather + fold on host
    y2 = np.empty((n, COUT, 16384), dtype=np.float32)
    for core in range(8):
        b, half = core // 2, core % 2
        r = res.results[core]["y2"]  # [P, 2, LSH] bf16
        y2[b, :, half * LSH:(half + 1) * LSH] = (
            r.transpose(1, 0, 2).reshape(COUT, LSH).astype(np.float32)
        )

    # bias epilogue (b1/b2 are zeros in this problem; exact otherwise)
    v = W2.astype(np.float64) @ b1.astype(np.float64) + b2.astype(np.float64)
    if np.any(v):
        y2 += v.astype(np.float32)[None, :, None]

    out = y2.reshape(n, c, 2, 2, 128, 128).transpose(0, 1, 4, 2, 5, 3)
    return np.ascontiguousarray(out).reshape(n, c, 256, 256)


# revision 36
# speedup vs baseline: 4.4442x; 1.0045x over previous
"""M2MRF module as a single fused-GEMM Bass kernel on 8 TRN2 NeuronCores.

The reference is two chained 1x1 convs with NO nonlinearity between them:
    y2 = W2 @ (W1 @ cols + b1) + b2 = (W2@W1) @ cols + (W2@b1 + b2)
so the device work collapses to one GEMM with the fused matrix
M = W2@W1 [256, 1024] (5.2x fewer FLOPs than the two-GEMM chain).

Math (per batch b of 4):
    cols = unfold(x[b], k=4, s=4)            # [1024, 16384]
    y2   = M @ cols                          # [256, 16384]
    out[b] = fold(y2 + v, k=2, s=2)          # [64, 256, 256], v = W2@b1+b2

Sharding: 8 cores = 4 batches x 2 L-halves (L = 16384 patch positions).
Each core runs one GEMM (256x1024x8192) in bf16 with fp32 PSUM
accumulation, streaming L-tiles of 512 through SBUF. Unfold/fold are
pure data movement and run on the host.
"""
import sys

sys.path.insert(0, "/opt/trn_rl_repo")

import numpy as np
import ml_dtypes

import concourse.bass as bass
import concourse.bacc as bacc
import concourse.mybir as mybir
import concourse.tile as tile
from concourse.bass_utils import run_bass_kernel_spmd

P = 128
NT = 512            # free-dim tile (one PSUM bank of fp32)
LSH = 8192          # L per core
NTILES = LSH // NT  # 16
KC = 8              # 1024 / 128 contraction chunks
COUT = 256
OGRP = 2            # L-tiles per output DMA

_BF16 = ml_dtypes.bfloat16


# Warmup row counts: climb the PE p-state ramp (0.65/1.2 GHz until 3us of
# continuous busy) on junk data, sized to end right as the first real
# operands land (~5.8us), with short-matmul cushion against a gap reset.
WARM_PLAN = [512] * 9 + [128] * 4

# Column widths per compute tile: narrow early tiles move the first matmul
# earlier (supply = 5.69 ns/col vs demand = 6.67 ns/col lets later tiles
# grow without starving the PE).
TILE_PLAN = [256, 256, 320, 384, 448, 384] + [512] * 10 + [448, 320, 256]
# Output flush groups (tile indices): ~1024-col groups, short final group so
# the drain after the last matmul is minimal.
OUT_GROUPS = [[0, 1, 2, 3], [4, 5], [6, 7], [8, 9], [10, 11], [12, 13], [14, 15], [16], [17], [18]]


def _build_nc():
    nc = bacc.Bacc("TRN2", target_bir_lowering=False)
    xc_dram = nc.dram_tensor("xc", [P, KC, LSH], mybir.dt.bfloat16, kind="ExternalInput")
    wt_dram = nc.dram_tensor("wt", [P, KC, COUT], mybir.dt.bfloat16, kind="ExternalInput")
    y2_dram = nc.dram_tensor("y2", [P, 2, LSH], mybir.dt.bfloat16, kind="ExternalOutput")

    starts = [sum(TILE_PLAN[:i]) for i in range(len(TILE_PLAN))]
    grp_of = {}
    for gi, g in enumerate(OUT_GROUPS):
        for t in g:
            grp_of[t] = gi

    with tile.TileContext(nc) as tc:
        with (
            tc.tile_pool(name="resident", bufs=1) as res,
            tc.tile_pool(name="xin", bufs=14) as xin,
            tc.tile_pool(name="outp", bufs=10) as outp,
            tc.tile_pool(name="ps", bufs=4, space="PSUM") as ps,
            tc.tile_pool(name="wps", bufs=1, space="PSUM") as wps,
        ):
            wt_sb = res.tile([P, KC, COUT], mybir.dt.bfloat16, tag="wt")

            # Dep-free warmup matmuls on a zeroed tile: climb the PE p-state
            # ramp before real data lands so real matmuls price at full clock.
            warm = res.tile([P, NT], mybir.dt.bfloat16, tag="warm")
            nc.vector.memset(warm[:], 0)
            wpt = wps.tile([P, NT], mybir.dt.float32, tag="wps")
            for rows in WARM_PLAN:
                nc.tensor.matmul(wpt[:, 0:rows], warm[:, 0:P], warm[:, 0:rows], start=True, stop=True)

            o_sb = None
            o_base = 0
            for ti, width in enumerate(TILE_PLAN):
                nsl = slice(starts[ti], starts[ti] + width)
                # DMA issue order tracks first use: w, xt0, xt1, ...
                if ti == 0:
                    nc.sync.dma_start(wt_sb[:], wt_dram.ap())
                xt = xin.tile([P, KC, NT], mybir.dt.bfloat16, tag="xt")
                nc.sync.dma_start(xt[:, :, 0:width], xc_dram.ap()[:, :, nsl])

                gi = grp_of[ti]
                g_tiles = OUT_GROUPS[gi]
                if ti == g_tiles[0]:
                    o_base = starts[ti]
                    g_width = sum(TILE_PLAN[t] for t in g_tiles)
                    o_sb = outp.tile([P, 2, 1536], mybir.dt.bfloat16, tag="o")
                last_grp = gi == len(OUT_GROUPS) - 1
                off = starts[ti] - o_base
                # On the final tile, do m2=1 first so the drain-gating last
                # copy is the faster DVE one.
                m2_order = (1, 0) if ti == len(TILE_PLAN) - 1 else (0, 1)
                for m2 in m2_order:
                    pt = ps.tile([P, NT], mybir.dt.float32, tag="ps")
                    for k in range(KC):
                        nc.tensor.matmul(
                            pt[:, 0:width],
                            wt_sb[:, k, m2 * P:(m2 + 1) * P],
                            xt[:, k, 0:width],
                            start=(k == 0),
                            stop=(k == KC - 1),
                        )
                    dst = o_sb[:, m2, off:off + width]
                    if m2 == 0:
                        nc.vector.tensor_copy(out=dst, in_=pt[:, 0:width])
                    else:
                        nc.scalar.copy(out=dst, in_=pt[:, 0:width])
                    if last_grp and ti == g_tiles[-1]:
                        # Tail trim: flush each m-half as soon as its copy
                        # lands, on the lighter HWDGE path (SP is idle here).
                        osl = slice(o_base, starts[ti] + width)
                        gw = starts[ti] + width - o_base
                        nc.sync.dma_start(
                            y2_dram.ap()[:, m2, osl], o_sb[:, m2, 0:gw]
                        )
                if not last_grp and ti == g_tiles[-1]:
                    osl = slice(o_base, starts[ti] + width)
                    gw = starts[ti] + width - o_base
                    nc.gpsimd.dma_start(y2_dram.ap()[:, :, osl], o_sb[:, :, 0:gw])

    nc.finalize()
    return nc


_NC_CACHE = None


def kernel(x, W1, b1, W2, b2):
    global _NC_CACHE
    x = np.asarray(x)
    W1, b1 = np.asarray(W1), np.asarray(b1)
    W2, b2 = np.asarray(W2), np.asarray(b2)
    n, c, h, w = x.shape  # 4, 64, 512, 512

    # ---- host: fuse the two pointwise convs into one matrix
    M = (W2.astype(np.float64) @ W1.astype(np.float64)).astype(np.float32)
    # wt[p, k, m] = M[m, k*128+p]
    wt = np.ascontiguousarray(
        M.T.astype(_BF16).reshape(KC, P, COUT).transpose(1, 0, 2)
    )

    # ---- host unfold: cols[b, c*16+kh*4+kw, ph*128+pw] = x[b,c,ph*4+kh,pw*4+kw]
    xb = x.astype(_BF16)
    cols = xb.reshape(n, c, 128, 4, 128, 4).transpose(0, 1, 3, 5, 2, 4)
    cols = np.ascontiguousarray(cols).reshape(n, 1024, 16384)

    if _NC_CACHE is None:
        _NC_CACHE = _build_nc()
    nc = _NC_CACHE

    in_maps = []
    for core in range(8):
        b, half = core // 2, core % 2
        xc = np.ascontiguousarray(
            cols[b].reshape(KC, P, 2 * LSH)[:, :, half * LSH:(half + 1) * LSH]
            .transpose(1, 0, 2)
        )
        in_maps.append({"xc": xc, "wt": wt})

    res = run_bass_kernel_spmd(nc, in_maps, core_ids=list(range(8)))

    # ---- gather + fold on host
    y2 = np.empty((n, COUT, 16384), dtype=np.float32)
    for core in range(8):
        b, half = core // 2, core % 2
        r = res.results[core]["y2"]  # [P, 2, LSH] bf16
        y2[b, :, half * LSH:(half + 1) * LSH] = (
            r.transpose(1, 0, 2).reshape(COUT, LSH).astype(np.float32)
        )

    # bias epilogue (b1/b2 are zeros in this problem; exact otherwise)
    v = W2.astype(np.float64) @ b1.astype(np.float64) + b2.astype(np.float64)
    if np.any(v):
        y2 += v.astype(np.float32)[None, :, None]

    out = y2.reshape(n, c, 2, 2, 128, 128).transpose(0, 1, 4, 2, 5, 3)
    return np.ascontiguousarray(out).reshape(n, c, 256, 256)


# revision 43
# speedup vs baseline: 4.5094x; 1.0147x over previous
"""M2MRF module as a single fused-GEMM Bass kernel on 8 TRN2 NeuronCores.

The reference is two chained 1x1 convs with NO nonlinearity between them:
    y2 = W2 @ (W1 @ cols + b1) + b2 = (W2@W1) @ cols + (W2@b1 + b2)
so the device work collapses to one GEMM with the fused matrix
M = W2@W1 [256, 1024] (5.2x fewer FLOPs than the two-GEMM chain).

Math (per batch b of 4):
    cols = unfold(x[b], k=4, s=4)            # [1024, 16384]
    y2   = M @ cols                          # [256, 16384]
    out[b] = fold(y2 + v, k=2, s=2)          # [64, 256, 256], v = W2@b1+b2

Sharding: 8 cores = 4 batches x 2 L-halves (L = 16384 patch positions).
Each core runs one GEMM (256x1024x8192) in bf16 with fp32 PSUM
accumulation, streaming column tiles through SBUF. Unfold/fold are pure
data movement and run on the host.

Schedule: the kernel is DMA-rate-bound (21.5 MB at ~360 GB/s = 59.7 us vs
54.6 us of matmul), so the end-to-end time is PE_start + compute + drain,
with DMA packed underneath. The early region k-splits narrow column groups
so the first matmul starts after only half the weights plus one narrow
slab (~4.3 us); warmup matmuls on junk data climb the PE p-state ramp
before that and fill predicted supply bubbles so real matmuls always price
at the full 2.4 GHz clock.
"""
import sys

sys.path.insert(0, "/opt/trn_rl_repo")

import numpy as np
import ml_dtypes

import concourse.bass as bass
import concourse.bacc as bacc
import concourse.mybir as mybir
import concourse.tile as tile
from concourse.bass_utils import run_bass_kernel_spmd

P = 128
NT = 512            # max free-dim tile (one PSUM bank of fp32)
LSH = 8192          # L per core
KC = 8              # 1024 / 128 contraction chunks
KH = KC // 2
COUT = 256

_BF16 = ml_dtypes.bfloat16

# Warmup row counts: climb the PE p-state ramp (0.65/1.2 GHz until 3us of
# continuous busy) on junk data, sized to end right as the first real
# operands land (~4.3us).
WARM_PLAN = [512] * 5 + [256] + [128] * 2

# Early region: EG column-groups of EW cols with K split in half; each
# half-piece transfers in 728ns and computes in ~856ns.
EG = 10
EW = 256
# Filler 128-row warmups before the given PE piece index, bridging predicted
# supply bubbles without letting the p-state ramp reset.
FILLERS = {2: 5}
# Late region: full-K column tiles.
LATE_PLAN = [512] * 10 + [256, 256]
# The scheduler launches a DMA as soon as its data deps are met, and the
# DMA queue is FIFO by request time — so an output flush that becomes ready
# before the last input gen (~22us) preempts input transfers and starves
# the PE. BIGFLUSH_THRU merges the early-region outputs with late tiles
# 0..BIGFLUSH_THRU into one flush whose readiness (~29us) lands safely
# after every input request is already queued.
BIGFLUSH_THRU = 0


def _build_nc():
    nc = bacc.Bacc("TRN2", target_bir_lowering=False)
    xc_dram = nc.dram_tensor("xc", [P, KC, LSH], mybir.dt.bfloat16, kind="ExternalInput")
    wt_dram = nc.dram_tensor("wt", [P, KC, COUT], mybir.dt.bfloat16, kind="ExternalInput")
    y2_dram = nc.dram_tensor("y2", [P, 2, LSH], mybir.dt.bfloat16, kind="ExternalOutput")

    with tile.TileContext(nc) as tc:
        with (
            tc.tile_pool(name="resident", bufs=1) as res,
            tc.tile_pool(name="xe", bufs=2 * EG) as xe,
            tc.tile_pool(name="xin", bufs=len(LATE_PLAN)) as xin,
            tc.tile_pool(name="oute", bufs=1) as oute,
            tc.tile_pool(name="outp", bufs=6) as outp,
            tc.tile_pool(name="ps", bufs=4, space="PSUM") as ps,
            tc.tile_pool(name="wps", bufs=1, space="PSUM") as wps,
        ):
            wt_sb = res.tile([P, KC, COUT], mybir.dt.bfloat16, tag="wt")

            # Dep-free warmup matmuls on a zeroed tile.
            warm = res.tile([P, NT], mybir.dt.bfloat16, tag="warm")
            nc.vector.memset(warm[:], 0)
            wpt = wps.tile([P, NT], mybir.dt.float32, tag="wps")
            for rows in WARM_PLAN:
                nc.tensor.matmul(wpt[:, 0:rows], warm[:, 0:P], warm[:, 0:rows], start=True, stop=True)

            def filler(n):
                for _ in range(n):
                    nc.tensor.matmul(wpt[:, 0:128], warm[:, 0:P], warm[:, 0:128], start=True, stop=True)

            # ---- early region DMAs: wA, g0a, g1a, wB, g0b, g1b, g2a, g2b...
            xep = {}

            def xdma(g, h):
                t = xe.tile([P, KH, EW], mybir.dt.bfloat16, tag="xe")
                csl = slice(g * EW, (g + 1) * EW)
                nc.sync.dma_start(t[:], xc_dram.ap()[:, h * KH:(h + 1) * KH, csl])
                xep[(g, h)] = t

            nc.sync.dma_start(wt_sb[:, 0:KH, :], wt_dram.ap()[:, 0:KH, :])
            xdma(0, 0)
            xdma(1, 0)
            nc.sync.dma_start(wt_sb[:, KH:KC, :], wt_dram.ap()[:, KH:KC, :])
            xdma(0, 1)
            xdma(1, 1)
            for g in range(2, EG):
                xdma(g, 0)
                xdma(g, 1)

            # ---- early region compute: g0a, g1a, [fill], g0b, g1b, g2a...
            pe_order = [(0, 0), (1, 0), (0, 1), (1, 1)]
            for g in range(2, EG):
                pe_order += [(g, 0), (g, 1)]

            base0 = EG * EW
            starts = [base0 + sum(LATE_PLAN[:i]) for i in range(len(LATE_PLAN))]
            big_w = base0 + sum(LATE_PLAN[:BIGFLUSH_THRU + 1])
            o_big = oute.tile([P, 2, big_w], mybir.dt.bfloat16, tag="oe")

            gps = {}
            for pi, (g, h) in enumerate(pe_order):
                if pi in FILLERS:
                    filler(FILLERS[pi])
                xt = xep[(g, h)]
                if h == 0:
                    gps[g] = [
                        ps.tile([P, EW], mybir.dt.float32, tag="ps", name=f"ps_g{g}m{m2}")
                        for m2 in range(2)
                    ]
                for m2 in range(2):
                    pt = gps[g][m2]
                    for kk in range(KH):
                        k = h * KH + kk
                        nc.tensor.matmul(
                            pt[:],
                            wt_sb[:, k, m2 * P:(m2 + 1) * P],
                            xt[:, kk, :],
                            start=(k == 0),
                            stop=(k == KC - 1),
                        )
                    if h == 1:
                        dst = o_big[:, m2, g * EW:(g + 1) * EW]
                        if m2 == 0:
                            nc.vector.tensor_copy(out=dst, in_=pt[:])
                        else:
                            nc.scalar.copy(out=dst, in_=pt[:])

            # ---- late region: full-K tiles. Tiles 0..BIGFLUSH_THRU join the
            # early outputs in o_big (one big deferred flush); the rest pair
            # into 1024-col groups, with the small final tile flushed per
            # m-half for a minimal drain.
            o_sb = None
            o_base = base0
            for ti, width in enumerate(LATE_PLAN):
                nsl = slice(starts[ti], starts[ti] + width)
                xt = xin.tile([P, KC, NT], mybir.dt.bfloat16, tag="xt")
                nc.sync.dma_start(xt[:, :, 0:width], xc_dram.ap()[:, :, nsl])
                last = ti == len(LATE_PLAN) - 1
                in_big = ti <= BIGFLUSH_THRU
                if in_big:
                    o_cur, off = o_big, starts[ti]
                else:
                    if (ti - BIGFLUSH_THRU) % 2 == 1:
                        o_sb = outp.tile([P, 2, 1024], mybir.dt.bfloat16, tag="o")
                        o_base = starts[ti]
                    o_cur, off = o_sb, starts[ti] - o_base
                m2_order = (1, 0) if last else (0, 1)
                for m2 in m2_order:
                    pt = ps.tile([P, NT], mybir.dt.float32, tag="ps")
                    for k in range(KC):
                        nc.tensor.matmul(
                            pt[:, 0:width],
                            wt_sb[:, k, m2 * P:(m2 + 1) * P],
                            xt[:, k, 0:width],
                            start=(k == 0),
                            stop=(k == KC - 1),
                        )
                    dst = o_cur[:, m2, off:off + width]
                    if m2 == 0:
                        nc.vector.tensor_copy(out=dst, in_=pt[:, 0:width])
                    else:
                        nc.scalar.copy(out=dst, in_=pt[:, 0:width])
                    if last:
                        # Flush each m-half of the small final tile as soon as
                        # its copy lands (SP is idle here).
                        osl = slice(o_base, starts[ti] + width)
                        gw = starts[ti] + width - o_base
                        nc.sync.dma_start(
                            y2_dram.ap()[:, m2, osl], o_sb[:, m2, 0:gw]
                        )
                if ti == BIGFLUSH_THRU:
                    nc.gpsimd.dma_start(y2_dram.ap()[:, :, 0:big_w], o_big[:])
                elif not last and not in_big and (ti - BIGFLUSH_THRU) % 2 == 0:
                    osl = slice(o_base, starts[ti] + width)
                    gw = starts[ti] + width - o_base
                    nc.gpsimd.dma_start(y2_dram.ap()[:, :, osl], o_sb[:, :, 0:gw])

    nc.finalize()
    return nc


_NC_CACHE = None


def kernel(x, W1, b1, W2, b2):
    global _NC_CACHE
    x = np.asarray(x)
    W1, b1 = np.asarray(W1), np.asarray(b1)
    W2, b2 = np.asarray(W2), np.asarray(b2)
    n, c, h, w = x.shape  # 4, 64, 512, 512

    # ---- host: fuse the two pointwise convs into one matrix
    M = (W2.astype(np.float64) @ W1.astype(np.float64)).astype(np.float32)
    # wt[p, k, m] = M[m, k*128+p]
    wt = np.ascontiguousarray(
        M.T.astype(_BF16).reshape(KC, P, COUT).transpose(1, 0, 2)
    )

    # ---- host unfold: cols[b, c*16+kh*4+kw, ph*128+pw] = x[b,c,ph*4+kh,pw*4+kw]
    xb = x.astype(_BF16)
    cols = xb.reshape(n, c, 128, 4, 128, 4).transpose(0, 1, 3, 5, 2, 4)
    cols = np.ascontiguousarray(cols).reshape(n, 1024, 16384)

    if _NC_CACHE is None:
        _NC_CACHE = _build_nc()
    nc = _NC_CACHE

    in_maps = []
    for core in range(8):
        b, half = core // 2, core % 2
        xc = np.ascontiguousarray(
            cols[b].reshape(KC, P, 2 * LSH)[:, :, half * LSH:(half + 1) * LSH]
            .transpose(1, 0, 2)
        )
        in_maps.append({"xc": xc, "wt": wt})

    res = run_bass_kernel_spmd(nc, in_maps, core_ids=list(range(8)))

    # ---- gather + fold on host
    y2 = np.empty((n, COUT, 16384), dtype=np.float32)
    for core in range(8):
        b, half = core // 2, core % 2
        r = res.results[core]["y2"]  # [P, 2, LSH] bf16
        y2[b, :, half * LSH:(half + 1) * LSH] = (
            r.transpose(1, 0, 2).reshape(COUT, LSH).astype(np.float32)
        )

    # bias epilogue (b1/b2 are zeros in this problem; exact otherwise)
    v = W2.astype(np.float64) @ b1.astype(np.float64) + b2.astype(np.float64)
    if np.any(v):
        y2 += v.astype(np.float32)[None, :, None]

    out = y2.reshape(n, c, 2, 2, 128, 128).transpose(0, 1, 4, 2, 5, 3)
    return np.ascontiguousarray(out).reshape(n, c, 256, 256)
